# revision 4
# baseline (speedup 1.0000x reference)
"""Two-branch attention (self + cross) Bass kernel for 8 trn2 NeuronCores.

Data-parallel over batch: B=8 batches, one per core.  Per core:
  qkv1 = x1 @ qkv_w       (q1, k1 head-transposed layout; v1 natural)
  k2,v2 from x2 @ qkv_w[:, 768:]
  branch1: softmax(q1 k1^T * sc) v1 @ proj_w + proj_b
  branch2: softmax(q1 k2^T * sc) v2 @ proj_w + proj_b

Implementation notes:
  - scoresT[k, q] computed directly (lhsT = kT slice, rhs = qT slice), so the
    softmax'd matrix is already transposed for the AV matmul; no max pass is
    needed because |score*scale| <= ~2 for these input scales.
  - The denominator sum_k exp is produced by a ones-column appended to v in
    the AV matmul (output row 64).  Normalization happens on o^T via
    reciprocal + rank-1 PE broadcast + DVE multiply.
  - All matmuls in bf16 with f32 PSUM accumulation.
"""

import numpy as np

import concourse.bass as bass
import concourse.mybir as mybir
from concourse import bacc
from concourse.tile import TileContext
from concourse.masks import make_identity
from concourse.bass_utils import run_bass_kernel_spmd

B, N, C = 8, 1024, 768
H, HD = 12, 64
NT = N // 128    # 8 token chunks
CK = C // 128    # 6 contraction chunks of C
QW = 512         # q processed in halves of 512
QJ = N // QW     # 2
SCALE = HD ** -0.5
F32 = mybir.dt.float32
BF16 = mybir.dt.bfloat16


def build(with_bias: bool):
    nc = bacc.Bacc("TRN2", target_bir_lowering=False, debug=False, num_devices=8)
    x1_e = nc.declare_dram_parameter("x1", [N, C], F32, isOutput=False)
    x2_e = nc.declare_dram_parameter("x2", [N, C], F32, isOutput=False)
    w_e = nc.declare_dram_parameter("qkv_w", [C, 3 * C], F32, isOutput=False)
    p_e = nc.declare_dram_parameter("proj_w", [C, C], F32, isOutput=False)
    pb_e = nc.declare_dram_parameter("proj_b", [C], F32, isOutput=False)
    o1_e = nc.declare_dram_parameter("out1", [N, C], F32, isOutput=True)
    o2_e = nc.declare_dram_parameter("out2", [N, C], F32, isOutput=True)

    with TileContext(nc) as tc:
        with (
            tc.tile_pool(name="persist", bufs=1) as pp,
            tc.tile_pool(name="tmp", bufs=2) as tp,
            tc.tile_pool(name="attn", bufs=4) as atp,
            tc.tile_pool(name="small", bufs=4) as smp,
            tc.tile_pool(name="psum", bufs=1, space="PSUM") as ps,
        ):
            # ---- constants ----
            ident = pp.tile([128, 128], F32, tag="ident")
            make_identity(nc, ident)
            ones_bf = pp.tile([1, 128], BF16, tag="ones_bf")
            nc.any.memset(ones_bf, 1.0)
            if with_bias:
                pb_f = pp.tile([1, C], F32, tag="pb_f")
                nc.sync.dma_start(pb_f[:], pb_e[None, :])
                pb_b = pp.tile([1, C], BF16, tag="pb_b")
                nc.vector.tensor_copy(pb_b[:], pb_f[:])

            # ---- weights: load f32, cast to bf16 ----
            Wb = [pp.tile([128, 3 * C], BF16, tag=f"Wb{r}", name=f"Wb{r}") for r in range(CK)]
            for r in range(CK):
                for s in range(3):
                    wt = tp.tile([128, C], F32, tag="w_ld")
                    nc.sync.dma_start(
                        wt[:], w_e[r * 128:(r + 1) * 128, s * C:(s + 1) * C]
                    )
                    nc.vector.tensor_copy(Wb[r][:, s * C:(s + 1) * C], wt[:])
            Pb = [pp.tile([128, C], BF16, tag=f"Pb{r}", name=f"Pb{r}") for r in range(CK)]
            for r in range(CK):
                wt = tp.tile([128, C], F32, tag="w_ld")
                nc.sync.dma_start(wt[:], p_e[r * 128:(r + 1) * 128, :])
                nc.vector.tensor_copy(Pb[r][:], wt[:])

            # ---- x load + PE transpose -> xT (bf16) ----
            xT = {}
            for name, x_e in (("x1", x1_e), ("x2", x2_e)):
                xT[name] = [pp.tile([128, N], BF16, tag=f"{name}T{c}", name=f"{name}T{c}") for c in range(CK)]
                for t in range(NT):
                    xt = tp.tile([128, C], F32, tag="x_ld")
                    nc.sync.dma_start(xt[:], x_e[t * 128:(t + 1) * 128, :])
                    for c in range(CK):
                        pt = ps.tile([128, 128], F32, tag="ps_a", bufs=4)
                        nc.tensor.transpose(pt[:], xt[:, c * 128:(c + 1) * 128], ident[:])
                        nc.vector.tensor_copy(
                            xT[name][c][:, t * 128:(t + 1) * 128], pt[:]
                        )

            # ---- qkv matmuls ----
            # q1,k1 transposed: rows 0:1536 of qkv(x1)^T ; k2: rows 768:1536 of qkv(x2)^T
            qk1T = [pp.tile([128, N], BF16, tag=f"qk1T{m}", name=f"qk1T{m}") for m in range(12)]
            k2T = [pp.tile([128, N], BF16, tag=f"k2T{m}", name=f"k2T{m}") for m in range(6)]

            def qkvT_chunk(dst, w_col0, src_xT, scale):
                # dst[:, :] = (x @ W[:, w_col0:w_col0+128])^T  (128 rows x N)
                for j in range(QJ):
                    pt = ps.tile([128, QW], F32, tag="ps_a", bufs=4)
                    for c in range(CK):
                        nc.tensor.matmul(
                            pt[:],
                            lhsT=Wb[c][:, w_col0:w_col0 + 128],
                            rhs=src_xT[c][:, j * QW:(j + 1) * QW],
                            start=(c == 0),
                            stop=(c == CK - 1),
                        )
                    if scale != 1.0:
                        nc.vector.tensor_scalar_mul(
                            dst[:, j * QW:(j + 1) * QW], pt[:], scale
                        )
                    else:
                        nc.vector.tensor_copy(dst[:, j * QW:(j + 1) * QW], pt[:])

            for m in range(12):
                qkvT_chunk(qk1T[m], m * 128, xT["x1"], SCALE if m < 6 else 1.0)
            for m in range(6):
                qkvT_chunk(k2T[m], C + m * 128, xT["x2"], 1.0)

            # v natural layout with ones column: [128, 12, 65] per token chunk
            vx = {}
            for name in ("x1", "x2"):
                vx[name] = [
                    pp.tile([128, H, HD + 1], BF16, tag=f"v_{name}_{t}", name=f"v_{name}_{t}")
                    for t in range(NT)
                ]
                for t in range(NT):
                    nc.any.memset(vx[name][t][:, :, HD], 1.0)
                    for n0, nw in ((0, 512), (512, 256)):
                        pt = ps.tile([128, nw], F32, tag="ps_a", bufs=4)
                        for c in range(CK):
                            nc.tensor.matmul(
                                pt[:],
                                lhsT=xT[name][c][:, t * 128:(t + 1) * 128],
                                rhs=Wb[c][:, 2 * C + n0:2 * C + n0 + nw],
                                start=(c == 0),
                                stop=(c == CK - 1),
                            )
                        h0, h1 = n0 // HD, (n0 + nw) // HD
                        nc.vector.tensor_copy(
                            vx[name][t][:, h0:h1, 0:HD],
                            pt[:].rearrange("p (h d) -> p h d", d=HD),
                        )

            # ---- attention + proj per branch ----
            oT = {}
            for br, (kT, vname) in enumerate(((qk1T, "x1"), (k2T, "x2"))):
                koff = 6 if br == 0 else 0  # k rows live at 768.. in qk1T
                v = vx[vname]
                oT[br] = [pp.tile([128, N], BF16, tag=f"oT{br}_{c}", name=f"oT{br}_{c}") for c in range(CK)]
                for h in range(H):
                    kt_tile = kT[koff + h // 2]
                    qt_tile = qk1T[h // 2]
                    r0 = (h % 2) * HD
                    for j in range(QJ):
                        po = ps.tile([HD + 1, QW], F32, tag="ps_o", bufs=2)
                        for c in range(NT):
                            pt = ps.tile([128, QW], F32, tag="ps_a", bufs=4)
                            nc.tensor.matmul(
                                pt[:],
                                lhsT=kt_tile[r0:r0 + HD, c * 128:(c + 1) * 128],
                                rhs=qt_tile[r0:r0 + HD, j * QW:(j + 1) * QW],
                                start=True,
                                stop=True,
                            )
                            at = atp.tile([128, QW], BF16, tag="at")
                            nc.scalar.activation(
                                at[:], pt[:], mybir.ActivationFunctionType.Exp
                            )
                            nc.tensor.matmul(
                                po[:],
                                lhsT=v[c][:, h, :],
                                rhs=at[:],
                                start=(c == 0),
                                stop=(c == NT - 1),
                            )
                        rec = smp.tile([1, QW], F32, tag="rec")
                        nc.vector.reciprocal(rec[:], po[HD:HD + 1, :])
                        recb = smp.tile([1, QW], BF16, tag="recb")
                        nc.vector.tensor_copy(recb[:], rec[:])
                        pb_ = ps.tile([HD, QW], F32, tag="ps_b", bufs=2)
                        nc.tensor.matmul(
                            pb_[:], lhsT=ones_bf[:, 0:HD], rhs=recb[:],
                            start=True, stop=True,
                        )
                        ot_un = atp.tile([HD, QW], F32, tag="ot_un")
                        nc.scalar.copy(ot_un[:], po[0:HD, :])
                        nc.vector.tensor_tensor(
                            oT[br][h // 2][r0:r0 + HD, j * QW:(j + 1) * QW],
                            ot_un[:],
                            pb_[:],
                            mybir.AluOpType.mult,
                        )

                # proj: out[t*128:+128, :] = o @ proj_w (+ proj_b)
                o_e = o1_e if br == 0 else o2_e
                for t in range(NT):
                    ot = tp.tile([128, C], F32, tag="out_sb")
                    for n0, nw in ((0, 384), (384, 384)):
                        pt = ps.tile([128, nw], F32, tag="ps_a", bufs=4)
                        for c in range(CK):
                            nc.tensor.matmul(
                                pt[:],
                                lhsT=oT[br][c][:, t * 128:(t + 1) * 128],
                                rhs=Pb[c][:, n0:n0 + nw],
                                start=(c == 0),
                                stop=(c == CK - 1) and not with_bias,
                            )
                        if with_bias:
                            nc.tensor.matmul(
                                pt[:], lhsT=ones_bf[:, 0:128],
                                rhs=pb_b[:, n0:n0 + nw],
                                start=False, stop=True,
                            )
                        nc.scalar.copy(ot[:, n0:n0 + nw], pt[:])
                    nc.sync.dma_start(o_e[t * 128:(t + 1) * 128, :], ot[:])

    nc.compile()
    return nc


_CACHE = {}


def _get_nc(with_bias: bool):
    if with_bias not in _CACHE:
        _CACHE[with_bias] = build(with_bias)
    return _CACHE[with_bias]


def kernel(x1, x2, qkv_w, proj_w, proj_b):
    x1 = np.ascontiguousarray(np.asarray(x1, dtype=np.float32))
    x2 = np.ascontiguousarray(np.asarray(x2, dtype=np.float32))
    qkv_w = np.ascontiguousarray(np.asarray(qkv_w, dtype=np.float32))
    proj_w = np.ascontiguousarray(np.asarray(proj_w, dtype=np.float32))
    proj_b = np.ascontiguousarray(np.asarray(proj_b, dtype=np.float32))

    with_bias = bool(np.any(proj_b))
    nc = _get_nc(with_bias)
    in_maps = [
        {"x1": x1[i], "x2": x2[i], "qkv_w": qkv_w, "proj_w": proj_w,
         "proj_b": proj_b}
        for i in range(B)
    ]
    res = run_bass_kernel_spmd(nc, in_maps, core_ids=list(range(B)))
    o1 = np.stack([res.results[i]["out1"] for i in range(B)])
    o2 = np.stack([res.results[i]["out2"] for i in range(B)])
    return (o1, o2)
